# revision 1
# baseline (speedup 1.0000x reference)
"""Multi-head causal attention (B=2, T=2048, C=1024, H=16, HS=64) on 8 TRN2
NeuronCores.

Sharding: 2 heads per core (tensor parallel). Each core receives the full
(pre-transposed) activations xT [B, C, T], its 2 heads' QKV weight slices
packed [C, 128], and its 128-column slice of w_proj transposed [128, C].
Each core computes a partial output [B, T, C]; the host sums the 8 partials
and adds b_proj.

Per-core kernel (all matmuls in float32r -- tf32-like, 1 cycle/row):
  - QT/KT/VT [128(2 heads x 64), T] via lhsT=weight chunks, rhs=xT chunks.
  - V_aug [keys, 128]: V (cols 0:64, via PE-transpose of VT) | ones (64:128).
  - Flash-style causal attention in transposed layout: S^T[keys, q] blocks
    via lhsT=KT block, rhs=QT slice; exp on ScalarE (no max subtraction --
    scores are O(1) by construction); O^T = [V|1].T @ P^T accumulated over
    key blocks gives both O rows (0:64) and the softmax sums l replicated
    (rows 64:128) in one pass.
  - Normalize with reciprocal_approx_fast + mixed-base tensor_tensor.
  - Output projection: lhsT = OhatT t-chunks, rhs = w_projT slice.

The two batches are software-pipelined: batch 1's QKV matmuls are emitted
between batch 0's attention groups so the PE always has independent work
while ScalarE (exp) catches up -- keeping the PE HAM clock at 2.4 GHz.
"""

import math
import sys
from contextlib import ExitStack

if "/opt/trn_rl_repo" not in sys.path:
    sys.path.insert(0, "/opt/trn_rl_repo")

import numpy as np

import concourse.mybir as mybir
import concourse.tile as tile
from concourse import bacc
from concourse.bass import ts
from concourse.bass_utils import run_bass_kernel_spmd
from concourse.tile_rust import add_dep_helper

B, T, C = 2, 2048, 1024
H, HS = 16, 64
NCORES = 8
HPC = H // NCORES  # heads per core
P = 128
G = 512  # q-group size
NG = T // G
KB = 128  # key block
NPO = C // P  # contraction chunks
F32 = mybir.dt.float32
F32R = mybir.dt.float32r
BF16 = mybir.dt.bfloat16

_nc_cache = {}


def _emit(tc):
    nc = tc.nc
    _last_pe = [None]
    xt = nc.dram_tensor("xt", [B, C, T], F32R, kind="ExternalInput").ap()
    wq2 = nc.dram_tensor("wq2", [C, 128], F32R, kind="ExternalInput").ap()
    wk2 = nc.dram_tensor("wk2", [C, 128], F32R, kind="ExternalInput").ap()
    wv2 = nc.dram_tensor("wv2", [C, 128], F32R, kind="ExternalInput").ap()
    wpt = nc.dram_tensor("wpt", [128, C], F32R, kind="ExternalInput").ap()
    tri = nc.dram_tensor("tri", [P, P], BF16, kind="ExternalInput").ap()
    identd = nc.dram_tensor("ident", [P, 64], BF16, kind="ExternalInput").ap()
    onesd = nc.dram_tensor("ones", [P, T // KB, 64], BF16, kind="ExternalInput").ap()
    out = nc.dram_tensor("out", [B, T, C], F32, kind="ExternalOutput").ap()

    ctx = ExitStack()
    persist = ctx.enter_context(tc.tile_pool(name="persist", bufs=1))
    xt_pool = ctx.enter_context(tc.tile_pool(name="xtp", bufs=4))
    qk_pool = ctx.enter_context(tc.tile_pool(name="qkp", bufs=2))
    vt_pool = ctx.enter_context(tc.tile_pool(name="vtp", bufs=2))
    vaug_pool = ctx.enter_context(tc.tile_pool(name="vaugp", bufs=2))
    pt_pool = ctx.enter_context(tc.tile_pool(name="ptp", bufs=4))
    norm_pool = ctx.enter_context(tc.tile_pool(name="normp", bufs=2))
    ohat_pool = ctx.enter_context(tc.tile_pool(name="ohatp", bufs=2))
    out_pool = ctx.enter_context(tc.tile_pool(name="outp", bufs=2))
    st_psum = ctx.enter_context(tc.tile_pool(name="stps", bufs=2, space="PSUM"))
    ot_psum = ctx.enter_context(tc.tile_pool(name="otps", bufs=2, space="PSUM"))
    mm_psum = ctx.enter_context(tc.tile_pool(name="mmps", bufs=2, space="PSUM"))

    wq_sb = persist.tile([P, NPO, 128], F32R, tag="wq")
    wk_sb = persist.tile([P, NPO, 128], F32R, tag="wk")
    wv_sb = persist.tile([P, NPO, 128], F32R, tag="wv")
    wpt_sb = persist.tile([P, C], F32R, tag="wpt")
    tri_sb = persist.tile([P, P], BF16, tag="tri")
    ident = persist.tile([P, 64], BF16, tag="ident")

    # ---- input loading: per-tg xT tiles, one 2MB DMA each ----
    def load_xt_tg(eng, b, tg, dep=None):
        t = xt_pool.tile([P, NPO, 512], F32R, tag="xt", name=f"xt{b}{tg}")
        i = eng.dma_start(
            t[:],
            xt[b, :, ts(tg, 512)].rearrange("(po pi) t -> pi po t", pi=P),
        )
        if dep is not None:
            add_dep_helper(i.ins, dep.ins, sync=True)
        return t, i

    nc.sync.dma_start(wq_sb[:], wq2.rearrange("(po pi) d -> pi po d", pi=P))
    nc.sync.dma_start(wk_sb[:], wk2.rearrange("(po pi) d -> pi po d", pi=P))
    nc.sync.dma_start(wv_sb[:], wv2.rearrange("(po pi) d -> pi po d", pi=P))
    nc.sync.dma_start(tri_sb[:], tri[:])
    nc.sync.dma_start(ident[:], identd[:])
    xt0, xt0_dmas = [], []
    for tg in range(NG):
        t, i = load_xt_tg(nc.sync, 0, tg, dep=xt0_dmas[tg - 2] if tg >= 2 else None)
        xt0.append(t)
        xt0_dmas.append(i)
        if tg == 1:
            nc.sync.dma_start(wpt_sb[:], wpt[:])

    def new_state(b, xts):
        st = {
            "b": b,
            "xt": xts,
            "qt": qk_pool.tile([P, T], F32R, tag="qt", name=f"qt{b}"),
            "kt": qk_pool.tile([P, T], F32R, tag="kt", name=f"kt{b}"),
            "vt": vt_pool.tile([P, T], BF16, tag="vt", name=f"vt{b}"),
            "ohat": ohat_pool.tile([P, T], F32R, tag="ohat", name=f"oh{b}"),
            "vaug": [],
        }
        for h in range(HPC):
            va = vaug_pool.tile(
                [P, T // KB, 128], BF16, tag=f"vaug{h}", name=f"va{b}{h}"
            )
            i = nc.gpsimd.dma_start(va[:, :, 64:128], onesd[:])
            add_dep_helper(i.ins, xt0_dmas[0].ins, sync=True)
            st["vaug"].append(va)
        return st

    def emit_qkv_group(st, which, tg):
        w_sb, dst = {
            "q": (wq_sb, st["qt"]),
            "k": (wk_sb, st["kt"]),
            "v": (wv_sb, st["vt"]),
        }[which]
        ps = mm_psum.tile([P, 512], F32, tag="mm", name=f"qkv{which}{tg}")
        for po in range(NPO):
            mi = nc.tensor.matmul(
                ps[:],
                w_sb[:, po, :],
                st["xt"][tg][:, po, :],
                start=(po == 0),
                stop=(po == NPO - 1),
            )
            _last_pe[0] = mi
        nc.vector.tensor_copy(dst[:, ts(tg, 512)], ps[:])

    def emit_vaug_part(st, kbg):
        vaug = st["vaug"]
        tps = [
            mm_psum.tile([P, 4, 64], BF16, tag="mm", name=f"vtr{h}")
            for h in range(HPC)
        ]
        for kk in range(4):
            kb = 4 * kbg + kk
            for h in range(HPC):
                nc.tensor.transpose(
                    tps[h][:, kk, :],
                    st["vt"][64 * h : 64 * h + 64, ts(kb, KB)],
                    ident[64 * h : 64 * h + 64, :],
                )
        for h in range(HPC):
            nc.vector.tensor_copy(
                vaug[h][:, 4 * kbg : 4 * kbg + 4, 0:64], tps[h][:]
            )

    def emit_attn_g(st, g):
        b, qt, kt, vaug, ohat = st["b"], st["qt"], st["kt"], st["vaug"], st["ohat"]
        l_sb = norm_pool.tile([P, G], F32, tag="lsb", name=f"l{b}{g}")
        rinv = norm_pool.tile([P, G], F32, tag="rinv", name=f"r{b}{g}")
        otps_h = [
            ot_psum.tile([P, G], F32, tag="ot", name=f"ot{h}") for h in range(HPC)
        ]
        n_j = 4 * g + 4
        for jg in range(math.ceil(n_j / 2)):
            js = [j for j in (2 * jg, 2 * jg + 1) if j < n_j]
            diag = 2 * jg >= 4 * g
            stps_h = [
                st_psum.tile([P, 2, G], F32, tag="st", name=f"st{h}")
                for h in range(HPC)
            ]
            pt_h = [
                pt_pool.tile([P, 2, G], BF16, tag="pt", name=f"pt{h}")
                for h in range(HPC)
            ]
            for idx, j in enumerate(js):
                r = j - 4 * g
                q0 = 128 * r if r >= 0 else 0
                for h in range(HPC):
                    hb = 64 * h
                    nc.tensor.matmul(
                        stps_h[h][:, idx, q0:G],
                        kt[hb : hb + 64, ts(j, KB)],
                        qt[hb : hb + 64, G * g + q0 : G * (g + 1)],
                        start=True,
                        stop=True,
                    )
            for h in range(HPC):
                stps, pt = stps_h[h], pt_h[h]
                if not diag:
                    nc.scalar.activation(
                        pt[:, :, :],
                        stps[:, :, :],
                        mybir.ActivationFunctionType.Exp,
                        scale=float(HS) ** -0.5,
                    )
                else:
                    for idx, j in enumerate(js):
                        q0 = 128 * (j - 4 * g)
                        nc.scalar.activation(
                            pt[:, idx, q0:G],
                            stps[:, idx, q0:G],
                            mybir.ActivationFunctionType.Exp,
                            scale=float(HS) ** -0.5,
                        )
                        nc.vector.tensor_tensor(
                            pt[:, idx, q0 : q0 + 128],
                            pt[:, idx, q0 : q0 + 128],
                            tri_sb[:],
                            mybir.AluOpType.mult,
                        )
            for idx, j in enumerate(js):
                r = j - 4 * g
                q0 = 128 * r if r >= 0 else 0
                for h in range(HPC):
                    nc.tensor.matmul(
                        otps_h[h][:, q0:G],
                        vaug[h][:, j, :],
                        pt_h[h][:, idx, q0:G],
                        start=(j == 0),
                        stop=(j == n_j - 1),
                    )
        stag = norm_pool.tile([P, G], F32, tag="stag", name=f"sg{b}{g}")
        for h in range(HPC):
            hb = 64 * h
            nc.vector.tensor_copy(stag[hb : hb + 64, :], otps_h[h][0:64, :])
            nc.vector.tensor_copy(l_sb[hb : hb + 64, :], otps_h[h][64:128, :])
        nc.vector.reciprocal_approx_fast(rinv[:], l_sb[:])
        nc.vector.tensor_tensor(
            ohat[:, ts(g, G)], stag[:], rinv[:], mybir.AluOpType.mult
        )

    def emit_proj(st, g):
        b, ohat = st["b"], st["ohat"]
        for tc4 in range(G // P):
            t0 = G * g + P * tc4
            o_sb = out_pool.tile([P, C], F32, tag="osb", name=f"osb{tc4}")
            for n in range(C // 512):
                pj = mm_psum.tile([P, 512], F32, tag="mm", name=f"pj{n}")
                nc.tensor.matmul(
                    pj[:],
                    ohat[:, t0 : t0 + P],
                    wpt_sb[:, ts(n, 512)],
                    start=True,
                    stop=True,
                )
                if (2 * tc4 + n) % 2 == 0:
                    nc.vector.tensor_copy(o_sb[:, ts(n, 512)], pj[:])
                else:
                    nc.scalar.copy(o_sb[:, ts(n, 512)], pj[:])
            nc.sync.dma_start(out[b, t0 : t0 + P, :], o_sb[:])

    # ================= pipelined emission =================
    st0 = new_state(0, xt0)
    st1 = new_state(1, [None] * NG)
    xt1_dmas = []
    for tg in range(NG):
        for which in ("q", "k", "v"):
            emit_qkv_group(st0, which, tg)
        emit_vaug_part(st0, tg)
        emit_attn_g(st0, tg)
        t, i = load_xt_tg(
            nc.gpsimd, 1, tg,
            dep=xt1_dmas[tg - 2] if tg >= 2 else xt0_dmas[NG - 1],
        )
        st1["xt"][tg] = t
        xt1_dmas.append(i)
        if tg >= 1:
            for which in ("q", "k", "v"):
                emit_qkv_group(st1, which, tg - 1)
    for which in ("q", "k", "v"):
        emit_qkv_group(st1, which, NG - 1)
    for g in range(NG):
        emit_vaug_part(st1, g)
        emit_attn_g(st1, g)
        emit_proj(st0, g)
        if g >= 1:
            emit_proj(st1, g - 1)
    emit_proj(st1, NG - 1)
    ctx.close()


def _build():
    if "nc" in _nc_cache:
        return _nc_cache["nc"]
    nc = bacc.Bacc("TRN2", target_bir_lowering=False, debug=False)
    with tile.TileContext(nc) as tc:
        _emit(tc)
    nc.compile()
    _nc_cache["nc"] = nc
    return nc


def _make_in_maps(x, wq, wk, wv, w_proj):
    xt = np.ascontiguousarray(x.transpose(0, 2, 1)).astype(np.float32)
    import ml_dtypes

    tri = np.triu(np.ones((P, P), dtype=np.float32)).astype(ml_dtypes.bfloat16)
    ident = np.tile(np.eye(64, dtype=np.float32), (2, 1)).astype(ml_dtypes.bfloat16)
    ones = np.ones((P, T // KB, 64), dtype=np.float32).astype(ml_dtypes.bfloat16)
    in_maps = []
    for c in range(NCORES):
        h0 = HPC * c
        in_maps.append(
            {
                "xt": xt,
                "wq2": np.ascontiguousarray(
                    np.concatenate([wq[h0 + i] for i in range(HPC)], axis=1)
                ).astype(np.float32),
                "wk2": np.ascontiguousarray(
                    np.concatenate([wk[h0 + i] for i in range(HPC)], axis=1)
                ).astype(np.float32),
                "wv2": np.ascontiguousarray(
                    np.concatenate([wv[h0 + i] for i in range(HPC)], axis=1)
                ).astype(np.float32),
                "wpt": np.ascontiguousarray(
                    w_proj[:, 128 * c : 128 * (c + 1)].T
                ).astype(np.float32),
                "tri": tri,
                "ident": ident,
                "ones": ones,
            }
        )
    return in_maps


def kernel(x, wq, wk, wv, w_proj, b_proj):
    x = np.asarray(x, dtype=np.float32)
    wq = np.asarray(wq, dtype=np.float32)
    wk = np.asarray(wk, dtype=np.float32)
    wv = np.asarray(wv, dtype=np.float32)
    w_proj = np.asarray(w_proj, dtype=np.float32)
    b_proj = np.asarray(b_proj, dtype=np.float32)

    nc = _build()
    in_maps = _make_in_maps(x, wq, wk, wv, w_proj)
    res = run_bass_kernel_spmd(nc, in_maps, core_ids=list(range(NCORES)))
    acc = np.zeros((B, T, C), dtype=np.float64)
    for r in res.results:
        acc += r["out"]
    return (acc + b_proj).astype(np.float32)



# revision 8
# speedup vs baseline: 1.2744x; 1.2744x over previous
"""Multi-head causal attention (B=2, T=2048, C=1024, H=16, HS=64) on 8 TRN2
NeuronCores.

Sharding: 2 heads per core (tensor parallel). Each core receives the full
(pre-transposed) activations xT [B, C, T], its 2 heads' QKV weight slices
packed [C, 128], and its 128-column slice of w_proj transposed [128, C].
Each core computes a partial output [B, T, C]; the host sums the 8 partials
and adds b_proj.

Per-core kernel (all matmuls in float32r -- tf32-like, 1 cycle/row):
  - QT/KT/VT [128(2 heads x 64), T] via lhsT=weight chunks, rhs=xT chunks.
  - V_aug [keys, 128]: V (cols 0:64, via PE-transpose of VT) | ones (64:128).
  - Flash-style causal attention in transposed layout: S^T[keys, q] blocks
    via lhsT=KT block, rhs=QT slice; exp on ScalarE (no max subtraction --
    scores are O(1) by construction); O^T = [V|1].T @ P^T accumulated over
    key blocks gives both O rows (0:64) and the softmax sums l replicated
    (rows 64:128) in one pass.
  - Normalize with reciprocal_approx_fast + mixed-base tensor_tensor.
  - Output projection: lhsT = OhatT t-chunks, rhs = w_projT slice.

The two batches are software-pipelined: batch 1's QKV matmuls are emitted
between batch 0's attention groups so the PE always has independent work
while ScalarE (exp) catches up -- keeping the PE HAM clock at 2.4 GHz.
"""

import math
import sys
from contextlib import ExitStack

if "/opt/trn_rl_repo" not in sys.path:
    sys.path.insert(0, "/opt/trn_rl_repo")

import numpy as np

import concourse.mybir as mybir
import concourse.tile as tile
from concourse import bacc
from concourse.bass import ts
from concourse.bass_utils import run_bass_kernel_spmd
from concourse.tile_rust import add_dep_helper

B, T, C = 2, 2048, 1024
H, HS = 16, 64
NCORES = 8
HPC = H // NCORES  # heads per core
P = 128
G = 512  # q-group size
NG = T // G
KB = 128  # key block
NPO = C // P  # contraction chunks
F32 = mybir.dt.float32
F32R = mybir.dt.float32r
BF16 = mybir.dt.bfloat16

_nc_cache = {}


def _emit(tc):
    nc = tc.nc
    _last_pe = [None]
    xt = nc.dram_tensor("xt", [B, C, T], BF16, kind="ExternalInput").ap()
    wq2 = nc.dram_tensor("wq2", [C, 128], BF16, kind="ExternalInput").ap()
    wk2 = nc.dram_tensor("wk2", [C, 128], BF16, kind="ExternalInput").ap()
    wv2 = nc.dram_tensor("wv2", [C, 128], BF16, kind="ExternalInput").ap()
    wpt = nc.dram_tensor("wpt", [128, C], BF16, kind="ExternalInput").ap()
    tri = nc.dram_tensor("tri", [P, P], BF16, kind="ExternalInput").ap()
    identd = nc.dram_tensor("ident", [P, 64], BF16, kind="ExternalInput").ap()
    onesd = nc.dram_tensor("ones", [P, T // KB, 64], BF16, kind="ExternalInput").ap()
    out = nc.dram_tensor("out", [B, T, C], F32, kind="ExternalOutput").ap()

    ctx = ExitStack()
    persist = ctx.enter_context(tc.tile_pool(name="persist", bufs=1))
    xt_pool = ctx.enter_context(tc.tile_pool(name="xtp", bufs=4))
    qk_pool = ctx.enter_context(tc.tile_pool(name="qkp", bufs=2))
    vt_pool = ctx.enter_context(tc.tile_pool(name="vtp", bufs=2))
    vaug_pool = ctx.enter_context(tc.tile_pool(name="vaugp", bufs=2))
    pt_pool = ctx.enter_context(tc.tile_pool(name="ptp", bufs=4))
    norm_pool = ctx.enter_context(tc.tile_pool(name="normp", bufs=2))
    ohat_pool = ctx.enter_context(tc.tile_pool(name="ohatp", bufs=2))
    out_pool = ctx.enter_context(tc.tile_pool(name="outp", bufs=2))
    st_psum = ctx.enter_context(tc.tile_pool(name="stps", bufs=2, space="PSUM"))
    ot_psum = ctx.enter_context(tc.tile_pool(name="otps", bufs=2, space="PSUM"))
    mm_psum = ctx.enter_context(tc.tile_pool(name="mmps", bufs=2, space="PSUM"))

    wq_sb = persist.tile([P, NPO, 128], BF16, tag="wq")
    wk_sb = persist.tile([P, NPO, 128], BF16, tag="wk")
    wv_sb = persist.tile([P, NPO, 128], BF16, tag="wv")
    wpt_sb = persist.tile([P, C], BF16, tag="wpt")
    tri_sb = persist.tile([P, P], BF16, tag="tri")
    ident = persist.tile([P, 64], BF16, tag="ident")

    # ---- input loading: per-tg xT tiles, one 2MB DMA each ----
    def load_xt_tg(eng, b, tg, dep=None):
        t = xt_pool.tile([P, NPO, 512], BF16, tag="xt", name=f"xt{b}{tg}")
        i = eng.dma_start(
            t[:],
            xt[b, :, ts(tg, 512)].rearrange("(po pi) t -> pi po t", pi=P),
        )
        if dep is not None:
            add_dep_helper(i.ins, dep.ins, sync=True)
        return t, i

    nc.scalar.dma_start(wq_sb[:], wq2.rearrange("(po pi) d -> pi po d", pi=P))
    nc.scalar.dma_start(wk_sb[:], wk2.rearrange("(po pi) d -> pi po d", pi=P))
    nc.scalar.dma_start(wv_sb[:], wv2.rearrange("(po pi) d -> pi po d", pi=P))
    nc.scalar.dma_start(tri_sb[:], tri[:])
    nc.scalar.dma_start(ident[:], identd[:])
    xt0, xt0_dmas = [], []
    for tg in range(NG):
        t, i = load_xt_tg(nc.sync, 0, tg, dep=xt0_dmas[tg - 2] if tg >= 2 else None)
        xt0.append(t)
        xt0_dmas.append(i)
        if tg == 1:
            nc.scalar.dma_start(wpt_sb[:], wpt[:])

    def new_state(b, xts):
        st = {
            "b": b,
            "xt": xts,
            "qt": qk_pool.tile([P, T], BF16, tag="qt", name=f"qt{b}"),
            "kt": qk_pool.tile([P, T], BF16, tag="kt", name=f"kt{b}"),
            "vt": vt_pool.tile([P, T], BF16, tag="vt", name=f"vt{b}"),
            "ohat": ohat_pool.tile([P, T], BF16, tag="ohat", name=f"oh{b}"),
            "vaug": [],
        }
        for h in range(HPC):
            va = vaug_pool.tile(
                [P, T // KB, 128], BF16, tag=f"vaug{h}", name=f"va{b}{h}"
            )
            i = nc.gpsimd.dma_start(va[:, :, 64:128], onesd[:])
            add_dep_helper(i.ins, xt0_dmas[0].ins, sync=True)
            st["vaug"].append(va)
        return st

    def emit_qkv_group(st, which, tg):
        w_sb, dst = {
            "q": (wq_sb, st["qt"]),
            "k": (wk_sb, st["kt"]),
            "v": (wv_sb, st["vt"]),
        }[which]
        ps = mm_psum.tile([P, 512], F32, tag="mm", name=f"qkv{which}{tg}")
        for po in range(NPO):
            mi = nc.tensor.matmul(
                ps[:],
                w_sb[:, po, :],
                st["xt"][tg][:, po, :],
                start=(po == 0),
                stop=(po == NPO - 1),
            )
            _last_pe[0] = mi
        nc.vector.tensor_copy(dst[:, ts(tg, 512)], ps[:])

    def emit_vaug_part(st, kbg):
        vaug = st["vaug"]
        tps = [
            mm_psum.tile([P, 4, 64], BF16, tag="mm", name=f"vtr{h}")
            for h in range(HPC)
        ]
        for kk in range(4):
            kb = 4 * kbg + kk
            for h in range(HPC):
                nc.tensor.transpose(
                    tps[h][:, kk, :],
                    st["vt"][64 * h : 64 * h + 64, ts(kb, KB)],
                    ident[64 * h : 64 * h + 64, :],
                )
        for h in range(HPC):
            nc.vector.tensor_copy(
                vaug[h][:, 4 * kbg : 4 * kbg + 4, 0:64], tps[h][:]
            )

    def emit_attn_g(st, g):
        b, qt, kt, vaug, ohat = st["b"], st["qt"], st["kt"], st["vaug"], st["ohat"]
        l_sb = norm_pool.tile([P, G], F32, tag="lsb", name=f"l{b}{g}")
        rinv = norm_pool.tile([P, G], F32, tag="rinv", name=f"r{b}{g}")
        otps_h = [
            ot_psum.tile([P, G], F32, tag="ot", name=f"ot{h}") for h in range(HPC)
        ]
        n_j = 4 * g + 4
        for jg in range(math.ceil(n_j / 2)):
            js = [j for j in (2 * jg, 2 * jg + 1) if j < n_j]
            diag = 2 * jg >= 4 * g
            stps_h = [
                st_psum.tile([P, 2, G], F32, tag="st", name=f"st{h}")
                for h in range(HPC)
            ]
            pt_h = [
                pt_pool.tile([P, 2, G], BF16, tag="pt", name=f"pt{h}")
                for h in range(HPC)
            ]
            for idx, j in enumerate(js):
                r = j - 4 * g
                q0 = 128 * r if r >= 0 else 0
                for h in range(HPC):
                    hb = 64 * h
                    nc.tensor.matmul(
                        stps_h[h][:, idx, q0:G],
                        kt[hb : hb + 64, ts(j, KB)],
                        qt[hb : hb + 64, G * g + q0 : G * (g + 1)],
                        start=True,
                        stop=True,
                    )
            for h in range(HPC):
                stps, pt = stps_h[h], pt_h[h]
                if not diag:
                    nc.scalar.activation(
                        pt[:, :, :],
                        stps[:, :, :],
                        mybir.ActivationFunctionType.Exp,
                        scale=float(HS) ** -0.5,
                    )
                else:
                    for idx, j in enumerate(js):
                        q0 = 128 * (j - 4 * g)
                        nc.scalar.activation(
                            pt[:, idx, q0:G],
                            stps[:, idx, q0:G],
                            mybir.ActivationFunctionType.Exp,
                            scale=float(HS) ** -0.5,
                        )
                        nc.vector.tensor_tensor(
                            pt[:, idx, q0 : q0 + 128],
                            pt[:, idx, q0 : q0 + 128],
                            tri_sb[:],
                            mybir.AluOpType.mult,
                        )
            for idx, j in enumerate(js):
                r = j - 4 * g
                q0 = 128 * r if r >= 0 else 0
                for h in range(HPC):
                    nc.tensor.matmul(
                        otps_h[h][:, q0:G],
                        vaug[h][:, j, :],
                        pt_h[h][:, idx, q0:G],
                        start=(j == 0),
                        stop=(j == n_j - 1),
                    )
        stag = norm_pool.tile([P, G], F32, tag="stag", name=f"sg{b}{g}")
        for h in range(HPC):
            hb = 64 * h
            nc.vector.tensor_copy(stag[hb : hb + 64, :], otps_h[h][0:64, :])
            nc.vector.tensor_copy(l_sb[hb : hb + 64, :], otps_h[h][64:128, :])
        nc.vector.reciprocal_approx_fast(rinv[:], l_sb[:])
        nc.vector.tensor_tensor(
            ohat[:, ts(g, G)], stag[:], rinv[:], mybir.AluOpType.mult
        )

    def emit_proj(st, g):
        b, ohat = st["b"], st["ohat"]
        for tc4 in range(G // P):
            t0 = G * g + P * tc4
            o_sb = out_pool.tile([P, C], F32, tag="osb", name=f"osb{tc4}")
            for n in range(C // 512):
                pj = mm_psum.tile([P, 512], F32, tag="mm", name=f"pj{n}")
                nc.tensor.matmul(
                    pj[:],
                    ohat[:, t0 : t0 + P],
                    wpt_sb[:, ts(n, 512)],
                    start=True,
                    stop=True,
                )
                if (2 * tc4 + n) % 2 == 0:
                    nc.vector.tensor_copy(o_sb[:, ts(n, 512)], pj[:])
                else:
                    nc.scalar.copy(o_sb[:, ts(n, 512)], pj[:])
            nc.sync.dma_start(out[b, t0 : t0 + P, :], o_sb[:])

    # ================= pipelined emission =================
    st0 = new_state(0, xt0)
    st1 = new_state(1, [None] * NG)
    xt1_dmas = []
    for tg in range(NG):
        for which in ("q", "k", "v"):
            emit_qkv_group(st0, which, tg)
        emit_vaug_part(st0, tg)
        emit_attn_g(st0, tg)
        t, i = load_xt_tg(
            nc.gpsimd, 1, tg,
            dep=xt1_dmas[tg - 2] if tg >= 2 else xt0_dmas[NG - 1],
        )
        st1["xt"][tg] = t
        xt1_dmas.append(i)
        if tg >= 1:
            for which in ("q", "k", "v"):
                emit_qkv_group(st1, which, tg - 1)
    for which in ("q", "k", "v"):
        emit_qkv_group(st1, which, NG - 1)
    for g in range(NG):
        emit_vaug_part(st1, g)
        emit_attn_g(st1, g)
        emit_proj(st0, g)
        if g >= 1:
            emit_proj(st1, g - 1)
    emit_proj(st1, NG - 1)
    ctx.close()


def _build():
    if "nc" in _nc_cache:
        return _nc_cache["nc"]
    nc = bacc.Bacc("TRN2", target_bir_lowering=False, debug=False)
    with tile.TileContext(nc) as tc:
        _emit(tc)
    nc.compile()
    _nc_cache["nc"] = nc
    return nc


def _make_in_maps(x, wq, wk, wv, w_proj):
    import ml_dtypes

    bf16 = ml_dtypes.bfloat16
    xt = np.ascontiguousarray(x.transpose(0, 2, 1)).astype(bf16)
    tri = np.triu(np.ones((P, P), dtype=np.float32)).astype(bf16)
    ident = np.tile(np.eye(64, dtype=np.float32), (2, 1)).astype(bf16)
    ones = np.ones((P, T // KB, 64), dtype=np.float32).astype(bf16)
    in_maps = []
    for c in range(NCORES):
        h0 = HPC * c
        in_maps.append(
            {
                "xt": xt,
                "wq2": np.ascontiguousarray(
                    np.concatenate([wq[h0 + i] for i in range(HPC)], axis=1)
                ).astype(bf16),
                "wk2": np.ascontiguousarray(
                    np.concatenate([wk[h0 + i] for i in range(HPC)], axis=1)
                ).astype(bf16),
                "wv2": np.ascontiguousarray(
                    np.concatenate([wv[h0 + i] for i in range(HPC)], axis=1)
                ).astype(bf16),
                "wpt": np.ascontiguousarray(
                    w_proj[:, 128 * c : 128 * (c + 1)].T
                ).astype(bf16),
                "tri": tri,
                "ident": ident,
                "ones": ones,
            }
        )
    return in_maps


def kernel(x, wq, wk, wv, w_proj, b_proj):
    x = np.asarray(x, dtype=np.float32)
    wq = np.asarray(wq, dtype=np.float32)
    wk = np.asarray(wk, dtype=np.float32)
    wv = np.asarray(wv, dtype=np.float32)
    w_proj = np.asarray(w_proj, dtype=np.float32)
    b_proj = np.asarray(b_proj, dtype=np.float32)

    nc = _build()
    in_maps = _make_in_maps(x, wq, wk, wv, w_proj)
    res = run_bass_kernel_spmd(nc, in_maps, core_ids=list(range(NCORES)))
    acc = np.zeros((B, T, C), dtype=np.float64)
    for r in res.results:
        acc += r["out"]
    return (acc + b_proj).astype(np.float32)

